# revision 9
# baseline (speedup 1.0000x reference)
"""Trainium2 Bass kernel for nn_DownBlock (PacConv1x1 -> PReLU -> Conv6x6s2 -> PReLU).

Math notes:
  - The PacConv2d adaptive kernel is exp(-0.5*||g-g||^2) == 1 exactly, so the
    guide tensor is mathematically unused: stage 1 is a plain 1x1 conv.
  - Stage 1: h[f,y,x] = prelu(sum_c pac_w[f,c] * x[c,y,x] + pac_b[f], alpha1)
  - Stage 2: 6x6 stride-2 conv with padding 2, + bias, prelu.

Implementation (per core, 2 of the 16 batch images, data-parallel over batch):
  - Stage 1 is a K=128 matmul per 512-position tile; the rhs access pattern
    picks x-parity phases so the epilogue (one fused Prelu ACT op) writes
    directly into the stage-2 input layout:
        Hx[(px, f), y+2, x//2 + 1]   (128 partitions, 132x66 image, zero halo)
  - Stage 2 (stride-2 6x6 conv) contracts (px, c) = 128 partitions per tap:
        out[o, i, j] = sum_{ky, n} Wp[ky,n][(px,c), o].T @ Hx[:, 2i+ky, j+n]
    = 18 accumulating K=128/M=64/N=512 matmuls per 8-row output block,
    then one fused Prelu ACT epilogue and a DMA out.
  - All matmul operands are float32r (TF32-class, full-rate on the PE).
"""
import numpy as np

import concourse.bacc as bacc
import concourse.mybir as mybir
from concourse.tile import TileContext
from concourse.bass_utils import run_bass_kernel_spmd
from concourse.masks import make_identity

F32 = mybir.dt.float32
F32R = mybir.dt.float32r
ALU = mybir.AluOpType

N_CORES = 8
B_TOTAL = 16
B_PER_CORE = B_TOTAL // N_CORES  # 2
CIN = 128
CG = 64  # guide channels (unused)
F = 64   # intermediate / output channels
H = W = 128
HO = WO = 64
K = 6
# phase image: rows 0..131 (y+2), cols 0..65 (x//2+1), zero halo
PR = 132
PC = 66

_CACHE = {}


def _build(repeat=1):
    """Build the Bass module.  repeat>1 re-emits the main pipeline that many
    times back-to-back (bench-only: lets wall-clock slope isolate per-pass
    device time from the ~0.5ms axon dispatch overhead)."""
    nc = bacc.Bacc("TRN2", target_bir_lowering=False, debug=False)

    x = nc.declare_dram_parameter("x", [B_PER_CORE, CIN, H, W], F32, isOutput=False)
    pac_w = nc.declare_dram_parameter("pac_w", [F, CIN], F32, isOutput=False)
    pac_b = nc.declare_dram_parameter("pac_b", [F], F32, isOutput=False)
    alpha1 = nc.declare_dram_parameter("alpha1", [1], F32, isOutput=False)
    conv_w = nc.declare_dram_parameter("conv_w", [F, F * K * K], F32, isOutput=False)
    conv_b = nc.declare_dram_parameter("conv_b", [F], F32, isOutput=False)
    alpha2 = nc.declare_dram_parameter("alpha2", [1], F32, isOutput=False)
    out = nc.declare_dram_parameter("out", [B_PER_CORE, F, HO, WO], F32, isOutput=True)

    PRELU = mybir.ActivationFunctionType.Prelu
    COPY = mybir.ActivationFunctionType.Copy

    with TileContext(nc) as tc:
        with (
            tc.tile_pool(name="const", bufs=1) as const,
            tc.tile_pool(name="xin", bufs=6) as xin,
            tc.tile_pool(name="hx", bufs=1) as hxp,
            tc.tile_pool(name="ob", bufs=3) as obp,
            tc.tile_pool(name="dv", bufs=3) as dvp,
            tc.tile_pool(name="psA", bufs=4, space="PSUM") as psA,
        ):
            # ---------------- constants / weight prep ----------------
            ident_f = const.tile([F, F], F32)
            make_identity(nc, ident_f[:])
            ident = const.tile([F, F], F32R)
            nc.vector.tensor_copy(ident[:], ident_f[:])

            # per-partition scalars (read against psum partitions 0:64)
            b1 = const.tile([F, 1], F32)
            b2 = const.tile([F, 1], F32)
            a1 = const.tile([F, 1], F32)
            a2 = const.tile([F, 1], F32)
            nc.sync.dma_start(out=b1[:], in_=pac_b[:, None])
            nc.sync.dma_start(out=b2[:], in_=conv_b[:, None])
            nc.sync.dma_start(out=a1[:], in_=alpha1.broadcast_to([F, 1]))
            nc.sync.dma_start(out=a2[:], in_=alpha2.broadcast_to([F, 1]))

            # staging of raw weights: [F(part), Cin] and [F(part), F*36]
            pac_stage = const.tile([F, CIN], F32R)
            w_stage = const.tile([F, F * K * K], F32R)
            nc.sync.dma_start(out=pac_stage[:], in_=pac_w[:].bitcast(F32R))
            nc.sync.dma_start(out=w_stage[:], in_=conv_w[:].bitcast(F32R))

            # pac_wT[c, f] = pac_w[f, c]: two 64-col transposes via matmul w/ identity
            pac_wT = const.tile([CIN, F], F32R)
            for half in range(2):
                pt = psA.tile([F, F], F32, tag="s2", name="pt")
                nc.tensor.matmul(
                    pt[:], pac_stage[:, half * 64:(half + 1) * 64], ident[:],
                    start=True, stop=True,
                )
                if half == 0:
                    nc.vector.tensor_copy(pac_wT[0:64, :], pt[:])
                else:
                    nc.scalar.activation(pac_wT[64:128, :], pt[:], COPY)

            # Wp[t18 = ky*3+n][(px, c), o] = conv_w[o, c, ky, 2n+px]
            wp = const.tile([CIN, 18 * F], F32R)
            for ky in range(K):
                for n in range(3):
                    t18 = ky * 3 + n
                    for px in range(2):
                        pt = psA.tile([F, F], F32, tag="s2", name="pt")
                        # lhsT: [o(64 part), c(64)] strided pick of tap (ky, 2n+px)
                        lhsT = w_stage[:, ky * K + 2 * n + px::K * K]
                        nc.tensor.matmul(pt[:], lhsT, ident[:], start=True, stop=True)
                        dst = wp[px * 64:(px + 1) * 64, t18 * F:(t18 + 1) * F]
                        if px == 0:
                            nc.vector.tensor_copy(dst, pt[:])
                        else:
                            nc.scalar.activation(dst, pt[:], COPY)

            # ---------------- phase tensors + halo zeroing ----------------
            zrow = const.tile([CIN, 2 * PC], F32)
            nc.gpsimd.memset(zrow[:], 0.0)

            hx = [
                hxp.tile([CIN, PR * PC], F32R, tag=f"hx{b}", name=f"hx{b}")
                for b in range(B_PER_CORE)
            ]
            for b in range(B_PER_CORE):
                t = hx[b]
                # top rows 0..1, bottom rows 130..131
                nc.vector.tensor_copy(t[:, 0:2 * PC], zrow[:])
                nc.vector.tensor_copy(t[:, 130 * PC:132 * PC], zrow[:])
                # left col 0 and right col 65 stripes (132 rows each)
                nc.vector.tensor_copy(t[:, 0:PR * PC:PC], zrow[:, 0:PR])
                nc.vector.tensor_copy(t[:, 65:PR * PC:PC], zrow[:, 0:PR])

            # ---------------- main pipeline ----------------
            for b in [bb for _ in range(repeat) for bb in range(B_PER_CORE)]:
                hxb = hx[b].rearrange("p (r c) -> p r c", c=PC)
                for t in range(8):  # 16-row x chunks
                    xt = xin.tile([CIN, 16 * W], F32R, tag="xt")
                    xtv = xt[:].rearrange("p (r c) -> p r c", r=16)
                    nc.sync.dma_start(
                        out=xtv,
                        in_=x[b, :, 16 * t:16 * t + 16, :].bitcast(F32R),
                    )
                    for h3 in range(2):  # 8-row halves
                        for px in range(2):  # x parity
                            ps = psA.tile([F, 8, 64], F32, tag="s1")
                            nc.tensor.matmul(
                                ps[:],
                                pac_wT[:],
                                xtv[:, 8 * h3:8 * h3 + 8, px::2],
                                start=True, stop=True,
                            )
                            # fused bias+prelu, writing the phase-image layout.
                            # px=0 runs on DVE (3 ops), px=1 on ACT (1 op with
                            # the +64 partition offset) to balance both engines.
                            r0 = 16 * t + 8 * h3 + 2
                            dst = hxb[px * 64:(px + 1) * 64, r0:r0 + 8, 1:65]
                            if px == 0:
                                t1 = dvp.tile([F, 8, 64], F32, tag="dv1", name="t1")
                                t2 = dvp.tile([F, 8, 64], F32, tag="dv2", name="t2")
                                nc.vector.tensor_scalar(
                                    t1[:], ps[:], b1[:], 0.0, ALU.add, ALU.max)
                                nc.vector.tensor_scalar(
                                    t2[:], ps[:], b1[:], 0.0, ALU.add, ALU.min)
                                nc.vector.scalar_tensor_tensor(
                                    dst, t2[:], a1[:], t1[:], ALU.mult, ALU.add)
                            else:
                                nc.scalar.activation(
                                    dst, ps[:], PRELU,
                                    bias=b1[:], scale=1.0, alpha=a1[:],
                                )
                    if t >= 1:
                        _stage2_block(nc, tc, psA, obp, hxb, wp, b2, a2, out, b, t - 1)
                _stage2_block(nc, tc, psA, obp, hxb, wp, b2, a2, out, b, 7)

    nc.compile()
    return nc


def _stage2_block(nc, tc, psA, obp, hxb, wp, b2, a2, out, b, ib):
    """18 accumulating taps -> prelu -> dma for output rows [8*ib, 8*ib+8)."""
    PRELU = mybir.ActivationFunctionType.Prelu
    ps = psA.tile([F, 8, 64], F32, tag="s2")
    for ky in range(K):
        for n in range(3):
            t18 = ky * 3 + n
            r0 = 16 * ib + ky
            rhs = hxb[:, r0:min(r0 + 16, PR):2, n:n + 64]
            nc.tensor.matmul(
                ps[:], wp[:, t18 * F:(t18 + 1) * F], rhs,
                start=(t18 == 0), stop=(t18 == 17),
            )
    ot = obp.tile([F, 8, 64], F32, tag="ot")
    nc.scalar.activation(ot[:], ps[:], PRELU, bias=b2[:], scale=1.0, alpha=a2[:])
    nc.sync.dma_start(out=out[b, :, 8 * ib:8 * ib + 8, :], in_=ot[:])


def _get_nc(repeat=1):
    key = f"nc{repeat}"
    if key not in _CACHE:
        _CACHE[key] = _build(repeat)
    return _CACHE[key]


def kernel(x, guide, pac_w, pac_b, alpha1, alpha2, conv_w, conv_b, **_unused):
    # guide is mathematically unused (adaptive kernel == exp(0) == 1)
    del guide
    x = np.ascontiguousarray(x, dtype=np.float32)
    shared = {
        "pac_w": np.ascontiguousarray(pac_w, dtype=np.float32).reshape(F, CIN),
        "pac_b": np.ascontiguousarray(pac_b, dtype=np.float32),
        "alpha1": np.ascontiguousarray(alpha1, dtype=np.float32),
        "conv_w": np.ascontiguousarray(conv_w, dtype=np.float32).reshape(F, F * K * K),
        "conv_b": np.ascontiguousarray(conv_b, dtype=np.float32),
        "alpha2": np.ascontiguousarray(alpha2, dtype=np.float32),
    }
    in_maps = [
        {"x": np.ascontiguousarray(x[i * B_PER_CORE:(i + 1) * B_PER_CORE]), **shared}
        for i in range(N_CORES)
    ]
    nc = _get_nc()
    res = run_bass_kernel_spmd(
        nc, in_maps, list(range(N_CORES)), trace=_CACHE.get("trace", False)
    )
    _CACHE["last_result"] = res
    return np.concatenate([r["out"] for r in res.results], axis=0)


# revision 17
# speedup vs baseline: 4.0094x; 4.0094x over previous
"""Trainium2 Bass kernel for nn_DownBlock (PacConv1x1 -> PReLU -> Conv6x6s2 -> PReLU).

Math notes:
  - The PacConv2d adaptive kernel is exp(-0.5*||g-g||^2) == 1 exactly, so the
    guide tensor is mathematically unused: stage 1 is a plain 1x1 conv.
  - Stage 1: h[f,y,x] = prelu(sum_c pac_w[f,c] * x[c,y,x] + pac_b[f], alpha1)
  - Stage 2: 6x6 stride-2 conv with padding 2, + bias, prelu.

Implementation (per core, 2 of the 16 batch images, data-parallel over batch):
  - Stage 1 is a K=128 float32r matmul per 512-position tile; the rhs access
    pattern picks x-parity phases so the epilogue (bias+prelu, split between
    ACT and DVE) writes directly into the stage-2 input layout:
        Hx[(px, f), y+2, x//2 + 1]   (128 partitions, 132x66 image, zero halo)
  - Stage 2 (stride-2 6x6 conv) contracts (px, c) = 128 partitions per tap:
        out[o, i, j] = sum_{ky, n} Wp[ky,n][(px,c), o].T @ Hx[:, 2i+ky, j+n]
    = 18 accumulating K=128/M=64/N=512 matmuls per 8-row output block, then a
    fused Prelu epilogue and a DMA out.
  - S2_BF16 selects the stage-2 operand dtype: float32r (TF32-class accuracy,
    M=64 fills the whole PE array) or bfloat16 (two blocks run concurrently in
    the two column halves of the PE array via tile_position -> ~2x stage-2
    matmul throughput at bf16 input rounding).
"""
import numpy as np

import concourse.bacc as bacc
import concourse.mybir as mybir
from concourse.tile import TileContext
from concourse.bass_utils import run_bass_kernel_spmd
from concourse.masks import make_identity

F32 = mybir.dt.float32
F32R = mybir.dt.float32r
BF16 = mybir.dt.bfloat16
FP16 = mybir.dt.float16
ALU = mybir.AluOpType

N_CORES = 8
B_TOTAL = 16
B_PER_CORE = B_TOTAL // N_CORES  # 2
CIN = 128
F = 64   # intermediate / output channels
H = W = 128
HO = WO = 64
K = 6
# phase image: rows 0..131 (y+2), cols 0..65 (x//2+1), zero halo
PR = 132
PC = 66

S2_BF16 = True  # stage-2 matmul dtype: False -> float32r, True -> float16

_CACHE = {}


def _build(repeat=1, s2_bf16=S2_BF16):
    """Build the Bass module.  repeat>1 re-emits the main pipeline that many
    times back-to-back (bench-only: lets wall-clock slope isolate per-pass
    device time from the multi-ms axon dispatch overhead)."""
    nc = bacc.Bacc("TRN2", target_bir_lowering=False, debug=False)

    x = nc.declare_dram_parameter("x", [B_PER_CORE, CIN, H, W], F32, isOutput=False)
    pac_w = nc.declare_dram_parameter("pac_w", [F, CIN], F32, isOutput=False)
    pac_b = nc.declare_dram_parameter("pac_b", [F], F32, isOutput=False)
    alpha1 = nc.declare_dram_parameter("alpha1", [1], F32, isOutput=False)
    conv_w = nc.declare_dram_parameter("conv_w", [F, F * K * K], F32, isOutput=False)
    conv_b = nc.declare_dram_parameter("conv_b", [F], F32, isOutput=False)
    alpha2 = nc.declare_dram_parameter("alpha2", [1], F32, isOutput=False)
    out = nc.declare_dram_parameter("out", [B_PER_CORE, F, HO, WO], F32, isOutput=True)

    PRELU = mybir.ActivationFunctionType.Prelu
    COPY = mybir.ActivationFunctionType.Copy
    DT2 = FP16 if s2_bf16 else F32R

    with TileContext(nc) as tc:
        with (
            tc.tile_pool(name="const", bufs=1) as const,
            tc.tile_pool(name="xin", bufs=8) as xin,
            tc.tile_pool(name="hx", bufs=1) as hxp,
            tc.tile_pool(name="ob", bufs=4) as obp,
            tc.tile_pool(name="dv", bufs=4) as dvp,
            tc.tile_pool(name="psA", bufs=4, space="PSUM") as psA,
        ):
            # ---------------- constants / weight prep ----------------
            ident_f = const.tile([F, F], F32)
            make_identity(nc, ident_f[:])
            ident = const.tile([F, F], F32R)
            nc.vector.tensor_copy(ident[:], ident_f[:])

            # per-partition scalars; stage-1 reads [0:64], the bf16 stage-2
            # epilogue reads all 128 (conv_b duplicated in both halves)
            b1 = const.tile([F, 1], F32)
            b2 = const.tile([CIN, 1], F32)
            a1 = const.tile([F, 1], F32)
            a2 = const.tile([CIN, 1], F32)
            nc.sync.dma_start(out=b1[:], in_=pac_b[:, None])
            nc.sync.dma_start(out=b2[0:64, :], in_=conv_b[:, None])
            nc.sync.dma_start(out=b2[64:128, :], in_=conv_b[:, None])
            nc.sync.dma_start(out=a1[:], in_=alpha1.broadcast_to([F, 1]))
            nc.sync.dma_start(out=a2[:], in_=alpha2.broadcast_to([CIN, 1]))

            # staging of raw weights: [F(part), Cin] and [F(part), F*36].
            # w_stage is staged in the stage-2 dtype (gpsimd DMA casts) so the
            # 36 tap transposes run at 1 cycle/row when DT2 is fp16.
            pac_stage = const.tile([F, CIN], F32R)
            w_stage = const.tile([F, F * K * K], DT2)
            nc.sync.dma_start(out=pac_stage[:], in_=pac_w[:].bitcast(F32R))
            if DT2 == F32R:
                nc.sync.dma_start(out=w_stage[:], in_=conv_w[:].bitcast(F32R))
            else:
                nc.gpsimd.dma_start(out=w_stage[:], in_=conv_w[:])
            ident2 = ident
            if DT2 != F32R:
                ident2 = const.tile([F, F], DT2, name="ident2")
                nc.vector.tensor_copy(ident2[:], ident_f[:])

            # pac_wT[c, f] = pac_w[f, c]: two 64-col transposes via matmul w/ identity
            pac_wT = const.tile([CIN, F], F32R)
            for half in range(2):
                pt = psA.tile([F, F], F32, tag="s2", name="pt")
                nc.tensor.matmul(
                    pt[:], pac_stage[:, half * 64:(half + 1) * 64], ident[:],
                    start=True, stop=True,
                )
                if half == 0:
                    nc.vector.tensor_copy(pac_wT[0:64, :], pt[:])
                else:
                    nc.scalar.activation(pac_wT[64:128, :], pt[:], COPY)

            # Wp[t18 = ky*3+n][(px, c), o] = conv_w[o, c, ky, 2n+px]
            wp = const.tile([CIN, 18 * F], DT2)
            for ky in range(K):
                for n in range(3):
                    t18 = ky * 3 + n
                    for px in range(2):
                        pt = psA.tile([F, F], F32, tag="s2", name="pt")
                        # lhsT: [o(64 part), c(64)] strided pick of tap (ky, 2n+px)
                        lhsT = w_stage[:, ky * K + 2 * n + px::K * K]
                        nc.tensor.matmul(pt[:], lhsT, ident2[:], start=True, stop=True)
                        dst = wp[px * 64:(px + 1) * 64, t18 * F:(t18 + 1) * F]
                        if px == 0:
                            nc.vector.tensor_copy(dst, pt[:])
                        else:
                            nc.scalar.activation(dst, pt[:], COPY)

            # ---------------- phase tensors + halo zeroing ----------------
            zrow = const.tile([CIN, 2 * PC], F32)
            nc.gpsimd.memset(zrow[:], 0.0)

            hx = [
                hxp.tile([CIN, PR * PC], DT2, tag=f"hx{b}", name=f"hx{b}")
                for b in range(B_PER_CORE)
            ]
            for b in range(B_PER_CORE):
                t = hx[b]
                # top rows 0..1, bottom rows 130..131
                nc.vector.tensor_copy(t[:, 0:2 * PC], zrow[:])
                nc.vector.tensor_copy(t[:, 130 * PC:132 * PC], zrow[:])
                # left col 0 and right col 65 stripes (132 rows each)
                nc.vector.tensor_copy(t[:, 0:PR * PC:PC], zrow[:, 0:PR])
                nc.vector.tensor_copy(t[:, 65:PR * PC:PC], zrow[:, 0:PR])

            # ---------------- main pipeline ----------------
            for b in [bb for _ in range(repeat) for bb in range(B_PER_CORE)]:
                hxb = hx[b].rearrange("p (r c) -> p r c", c=PC)
                for t in range(8):  # 16-row x chunks
                    xt = xin.tile([CIN, 16 * W], F32R, tag="xt")
                    xtv = xt[:].rearrange("p (r c) -> p r c", r=16)
                    # two half-chunk DMAs: h3=0 matmuls start after only 8 rows
                    nc.sync.dma_start(
                        out=xtv[:, 0:8],
                        in_=x[b, :, 16 * t:16 * t + 8, :].bitcast(F32R),
                    )
                    nc.sync.dma_start(
                        out=xtv[:, 8:16],
                        in_=x[b, :, 16 * t + 8:16 * t + 16, :].bitcast(F32R),
                    )
                    for h3 in range(2):  # 8-row halves
                        for px in range(2):  # x parity
                            ps = psA.tile([F, 8, 64], F32, tag="s1")
                            nc.tensor.matmul(
                                ps[:],
                                pac_wT[:],
                                xtv[:, 8 * h3:8 * h3 + 8, px::2],
                                start=True, stop=True,
                            )
                            # fused bias+prelu, writing the phase-image layout.
                            # px=0 runs on DVE (3 ops), px=1 on ACT (1 op with
                            # the +64 partition offset) to balance both engines.
                            r0 = 16 * t + 8 * h3 + 2
                            dst = hxb[px * 64:(px + 1) * 64, r0:r0 + 8, 1:65]
                            if px == 0:
                                t1 = dvp.tile([F, 8, 64], F32, tag="dv1", name="t1")
                                t2 = dvp.tile([F, 8, 64], F32, tag="dv2", name="t2")
                                nc.vector.tensor_scalar(
                                    t1[:], ps[:], b1[:], 0.0, ALU.add, ALU.max)
                                nc.vector.tensor_scalar(
                                    t2[:], ps[:], b1[:], 0.0, ALU.add, ALU.min)
                                nc.vector.scalar_tensor_tensor(
                                    dst, t2[:], a1[:], t1[:], ALU.mult, ALU.add)
                            else:
                                nc.scalar.activation(
                                    dst, ps[:], PRELU,
                                    bias=b1[:], scale=1.0, alpha=a1[:],
                                )
                    if s2_bf16:
                        # paired blocks: pair s ready after chunk 2s+2
                        if t >= 2 and t % 2 == 0:
                            _s2_pair(nc, psA, obp, hxb, wp, b2, a2, out, b, (t - 2) // 2)
                    else:
                        if t >= 1:
                            _s2_single(nc, psA, obp, hxb, wp, b2, a2, out, b, t - 1)
                if s2_bf16:
                    _s2_pair(nc, psA, obp, hxb, wp, b2, a2, out, b, 3)
                else:
                    _s2_single(nc, psA, obp, hxb, wp, b2, a2, out, b, 7)

    nc.compile()
    return nc


def _s2_single(nc, psA, obp, hxb, wp, b2, a2, out, b, ib):
    """float32r: 18 accumulating taps -> prelu -> dma, output rows [8ib, 8ib+8)."""
    PRELU = mybir.ActivationFunctionType.Prelu
    ps = psA.tile([F, 8, 64], F32, tag="s2", name="ps")
    for ky in range(K):
        for n in range(3):
            t18 = ky * 3 + n
            r0 = 16 * ib + ky
            rhs = hxb[:, r0:min(r0 + 16, PR):2, n:n + 64]
            nc.tensor.matmul(
                ps[:], wp[:, t18 * F:(t18 + 1) * F], rhs,
                start=(t18 == 0), stop=(t18 == 17),
            )
    ot = obp.tile([F, 8, 64], F32, tag="ot", name="ot")
    nc.scalar.activation(ot[:], ps[:], PRELU, bias=b2[0:64, :], scale=1.0,
                         alpha=a2[0:64, :])
    nc.sync.dma_start(out=out[b, :, 8 * ib:8 * ib + 8, :], in_=ot[:])


def _s2_pair(nc, psA, obp, hxb, wp, b2, a2, out, b, s):
    """bf16: blocks 2s and 2s+1 run concurrently in the two column halves of
    the PE array (tile_position), accumulating into one PSUM bank."""
    PRELU = mybir.ActivationFunctionType.Prelu
    ps = psA.tile([CIN, 8, 64], F32, tag="s2", name="ps")
    for ky in range(K):
        for n in range(3):
            t18 = ky * 3 + n
            for half in range(2):
                ib = 2 * s + half
                r0 = 16 * ib + ky
                rhs = hxb[:, r0:min(r0 + 16, PR):2, n:n + 64]
                nc.tensor.matmul(
                    ps[half * 64:(half + 1) * 64], wp[:, t18 * F:(t18 + 1) * F],
                    rhs, start=(t18 == 0), stop=(t18 == 17),
                    tile_position=(0, half * 64),
                )
    ot = obp.tile([CIN, 8, 64], F32, tag="ot", name="ot")
    nc.scalar.activation(ot[:], ps[:], PRELU, bias=b2[:], scale=1.0, alpha=a2[:])
    for half in range(2):
        ib = 2 * s + half
        nc.sync.dma_start(
            out=out[b, :, 8 * ib:8 * ib + 8, :],
            in_=ot[half * 64:(half + 1) * 64],
        )


def _get_nc(repeat=1, s2_bf16=S2_BF16):
    key = f"nc{repeat}_{s2_bf16}"
    if key not in _CACHE:
        _CACHE[key] = _build(repeat, s2_bf16)
    return _CACHE[key]


def kernel(x, guide, pac_w, pac_b, alpha1, alpha2, conv_w, conv_b, **_unused):
    # guide is mathematically unused (adaptive kernel == exp(0) == 1)
    del guide
    x = np.ascontiguousarray(x, dtype=np.float32)
    shared = {
        "pac_w": np.ascontiguousarray(pac_w, dtype=np.float32).reshape(F, CIN),
        "pac_b": np.ascontiguousarray(pac_b, dtype=np.float32),
        "alpha1": np.ascontiguousarray(alpha1, dtype=np.float32),
        "conv_w": np.ascontiguousarray(conv_w, dtype=np.float32).reshape(F, F * K * K),
        "conv_b": np.ascontiguousarray(conv_b, dtype=np.float32),
        "alpha2": np.ascontiguousarray(alpha2, dtype=np.float32),
    }
    in_maps = [
        {"x": np.ascontiguousarray(x[i * B_PER_CORE:(i + 1) * B_PER_CORE]), **shared}
        for i in range(N_CORES)
    ]
    nc = _get_nc()
    # The first execution of a freshly loaded NEFF occasionally trips an
    # NRT_EXEC_UNIT_UNRECOVERABLE in the runtime; a straight retry succeeds.
    last_exc = None
    for _ in range(3):
        try:
            res = run_bass_kernel_spmd(
                nc, in_maps, list(range(N_CORES)), trace=_CACHE.get("trace", False)
            )
            break
        except Exception as exc:  # noqa: BLE001
            last_exc = exc
    else:
        raise last_exc
    _CACHE["last_result"] = res
    return np.concatenate([r["out"] for r in res.results], axis=0)
